# revision 5
# baseline (speedup 1.0000x reference)
"""MF2Net Trainium2 Bass kernel — self-contained.

Strategy (data-parallel over batch, 8 NeuronCores):
  - Host: shard batch 16384 -> 8 x 2048, transpose x to [4096, 2048] per core so
    the device streams contraction-major tiles with no on-chip transpose.
  - Device: everything lives transposed [feature_partitions, batch_free].
    Big matmuls: h1 = relu(x @ W1 + b1) via 32 K-chunks accumulating in PSUM.
    2-class softmax == sigmoid(l0 - l1): the last base-learner matmul emits
    [d; -d] (2-col lhsT), so sigmoid gives both classes and is_le(d2_e, d2_a)
    gives the exact Choquet comparisons in fp32.
  - Fuzzy nets (single+comp) fused into one 128-wide hidden layer; the Pe1/Pa1
    feature columns are folded into weights/bias (Pe1 = 1 - Pe0).
  - correct = lab + (1-2*lab) * 1[d >= 0] with both label vectors host-packed.
  - Output written as [2, 2048] per core; host transposes/concats.
"""
import sys

if "/opt/trn_rl_repo" not in sys.path:
    sys.path.insert(0, "/opt/trn_rl_repo")

import numpy as np

_NCORES = 8
_B = 16384
_BL = _B // _NCORES      # 2048 rows per core
_KD = 4096               # eeg/audio input dim
_NB = _BL // 512         # 4 psum column chunks
_KC = _KD // 128         # 32 contraction chunks

_CACHE = {}


def _layout():
    """spack column layout: name -> (rows, col_off, col_cnt). Mirrored host/device."""
    cols = {}
    off = 0

    def add(name, rows, n):
        nonlocal off
        cols[name] = (rows, off, n)
        off += n

    add("W2e", 128, 64); add("W2a", 128, 64)          # extractor L2 [128 -> 64]
    add("BW1e", 64, 128); add("BW1a", 64, 128)        # base learner L1 [64 -> 128]
    add("BW2e", 128, 64); add("BW2a", 128, 64)        # base learner L2 [128 -> 64]
    add("W3d2e", 64, 2); add("W3d2a", 64, 2)          # BL L3 -> [d, -d]
    add("W2s1", 128, 2); add("W2s2", 128, 2)          # single L2 cols (0,1) / (2,3), zero rows 64:
    add("W2cp", 128, 2)                                # comp L2, zero rows :64
    for r in range(4):                                 # fuzzy L1 rows (K=1 lhsT): ce, Pe0, ca, Pa0
        add(f"U1r{r}", 1, 128)
    add("pm1", 2, 1)                                   # [+1; -1]
    add("b1e", 128, 1); add("b1a", 128, 1)
    add("b2e", 64, 1); add("b2a", 64, 1)
    add("Bb1e", 128, 1); add("Bb1a", 128, 1)
    add("Bb2e", 64, 1); add("Bb2a", 64, 1)
    add("b3d2e", 2, 1); add("b3d2a", 2, 1)
    add("b1cat", 128, 1)
    add("b2s1", 2, 1); add("b2s2", 2, 1); add("b2cp", 2, 1)
    return cols, off


def _build_module(mm_dtype="float32"):
    import concourse.tile as tile
    from concourse import bacc, mybir

    f32 = mybir.dt.float32
    fmm = getattr(mybir.dt, mm_dtype)
    AF = mybir.ActivationFunctionType
    ALU = mybir.AluOpType
    cols, S = _layout()

    nc = bacc.Bacc(None, target_bir_lowering=False)
    xe = nc.dram_tensor("xe", [_KD, _BL], fmm, kind="ExternalInput")
    xa = nc.dram_tensor("xa", [_KD, _BL], fmm, kind="ExternalInput")
    w1e = nc.dram_tensor("w1e", [128, _KD], fmm, kind="ExternalInput")
    w1a = nc.dram_tensor("w1a", [128, _KD], fmm, kind="ExternalInput")
    lab12 = nc.dram_tensor("lab12", [1, 2 * _BL], f32, kind="ExternalInput")  # [lab | 1-2*lab]
    spk = nc.dram_tensor("spk", [128, S], f32, kind="ExternalInput")
    out = nc.dram_tensor("out", [2, _BL], f32, kind="ExternalOutput")

    J = [(j, slice(j * 512, (j + 1) * 512)) for j in range(_NB)]

    with tile.TileContext(nc) as tc:
        with (
            tc.tile_pool(name="const", bufs=1) as cpool,
            tc.tile_pool(name="xin", bufs=3) as xpool,
            tc.tile_pool(name="keep", bufs=1) as keep,
            tc.tile_pool(name="chain", bufs=2) as chain,
            tc.tile_pool(name="tail", bufs=2) as tail,
            tc.tile_pool(name="psA", bufs=4, space="PSUM") as psA,
            tc.tile_pool(name="psB", bufs=4, space="PSUM") as psB,
        ):
            spk_sb = cpool.tile([128, S], f32)
            nc.sync.dma_start(spk_sb[:], spk[:])

            def P(name):
                r, o, n = cols[name]
                return spk_sb[0:r, o:o + n]

            w1sb = {}
            for m, wd in (("e", w1e), ("a", w1a)):
                t = cpool.tile([128, _KD], fmm, name=f"w1{m}_sb")
                nc.sync.dma_start(t[:], wd[:])
                w1sb[m] = t
            lab_sb = cpool.tile([1, 2 * _BL], f32, name="lab_sb")
            nc.sync.dma_start(lab_sb[:], lab12[:])

            d2 = {}
            PT = {}
            for m, xd in (("e", xe), ("a", xa)):
                h1ps = [psA.tile([128, 512], f32, tag="h1", name=f"h1ps_{m}{j}")
                        for j in range(_NB)]
                for k in range(_KC):
                    xt = xpool.tile([128, _BL], fmm, tag="x", name=f"xt_{m}{k}")
                    nc.sync.dma_start(xt[:], xd[k * 128:(k + 1) * 128, :])
                    for j, js in J:
                        nc.tensor.matmul(
                            h1ps[j][:], w1sb[m][:, k * 128:(k + 1) * 128], xt[:, js],
                            start=(k == 0), stop=(k == _KC - 1))

                h1 = chain.tile([128, _BL], f32, tag="hb", name=f"h1_{m}")
                for j, js in J:
                    nc.scalar.activation(out=h1[:, js], in_=h1ps[j][:],
                                         func=AF.Relu, bias=P("b1" + m), scale=1.0)
                h2 = chain.tile([64, _BL], f32, tag="hs", name=f"h2_{m}")
                for j, js in J:
                    ps = psB.tile([128, 512], f32, tag="ps2", name=f"h2ps_{m}{j}")
                    nc.tensor.matmul(ps[0:64, :], P("W2" + m), h1[:, js],
                                     start=True, stop=True)
                    nc.scalar.activation(out=h2[:, js], in_=ps[0:64, :],
                                         func=AF.Identity, bias=P("b2" + m), scale=1.0)
                g1 = chain.tile([128, _BL], f32, tag="hb", name=f"g1_{m}")
                for j, js in J:
                    ps = psB.tile([128, 512], f32, tag="ps2", name=f"g1ps_{m}{j}")
                    nc.tensor.matmul(ps[:], P("BW1" + m), h2[:, js],
                                     start=True, stop=True)
                    nc.scalar.activation(out=g1[:, js], in_=ps[:],
                                         func=AF.Relu, bias=P("Bb1" + m), scale=1.0)
                g2 = chain.tile([64, _BL], f32, tag="hs", name=f"g2_{m}")
                for j, js in J:
                    ps = psB.tile([128, 512], f32, tag="ps2", name=f"g2ps_{m}{j}")
                    nc.tensor.matmul(ps[0:64, :], P("BW2" + m), g1[:, js],
                                     start=True, stop=True)
                    nc.scalar.activation(out=g2[:, js], in_=ps[0:64, :],
                                         func=AF.Relu, bias=P("Bb2" + m), scale=1.0)
                # d2 = [l0-l1; l1-l0] (bias folded); P = sigmoid(d2) = both softmax classes
                d2m = keep.tile([2, _BL], f32, name=f"d2_{m}")
                for j, js in J:
                    ps = psB.tile([128, 512], f32, tag="ps2", name=f"d2ps_{m}{j}")
                    nc.tensor.matmul(ps[0:2, :], P("W3d2" + m), g2[:, js],
                                     start=True, stop=True)
                    nc.scalar.activation(out=d2m[:, js], in_=ps[0:2, :],
                                         func=AF.Identity, bias=P("b3d2" + m), scale=1.0)
                d2[m] = d2m
                Pm = keep.tile([2, _BL], f32, name=f"P_{m}")
                nc.scalar.activation(out=Pm[:], in_=d2m[:], func=AF.Sigmoid,
                                     bias=0.0, scale=1.0)
                PT[m] = Pm

            # ---- tail: per 512-column chunk ----
            for j, js in J:
                # correct_m = lab1 + lab2 * 1[d_m >= 0]
                cr = {}
                for m in ("e", "a"):
                    c = tail.tile([1, 512], f32, name=f"cr{m}")
                    nc.vector.tensor_scalar(out=c[:], in0=d2[m][0:1, js], scalar1=0.0,
                                            scalar2=None, op0=ALU.is_ge)
                    nc.vector.tensor_tensor(out=c[:], in0=c[:], in1=lab_sb[0:1, _BL + j * 512:_BL + (j + 1) * 512],
                                            op=ALU.mult)
                    nc.vector.tensor_tensor(out=c[:], in0=c[:], in1=lab_sb[0:1, js],
                                            op=ALU.add)
                    cr[m] = c

                # fuzzy L1 (single+comp fused, hidden 128): 4 accumulating K=1 matmuls
                srcs = [cr["e"][:], PT["e"][0:1, js], cr["a"][:], PT["a"][0:1, js]]
                u1ps = psB.tile([128, 512], f32, tag="ps2", name=f"u1ps{j}")
                for r, src in enumerate(srcs):
                    nc.tensor.matmul(u1ps[:], P(f"U1r{r}"), src,
                                     start=(r == 0), stop=(r == 3))
                u1 = tail.tile([128, 512], f32, name="u1t")
                nc.scalar.activation(out=u1[:], in_=u1ps[:], func=AF.Relu,
                                     bias=P("b1cat"), scale=1.0)

                pair = {}
                for nm, wn, bn in (("mu1", "W2s1", "b2s1"), ("mu2", "W2s2", "b2s2"),
                                   ("cc", "W2cp", "b2cp")):
                    t = tail.tile([2, 512], f32, name=f"pr_{nm}")
                    ps = psB.tile([128, 512], f32, tag="ps2", name=f"{nm}ps{j}")
                    nc.tensor.matmul(ps[0:2, :], P(wn), u1[:], start=True, stop=True)
                    nc.scalar.activation(out=t[:], in_=ps[0:2, :], func=AF.Sigmoid,
                                         bias=P(bn), scale=1.0)
                    pair[nm] = t

                # Choquet: ind = 1[Pe <= Pa] == 1[d2_e <= d2_a] rowwise
                ind = tail.tile([2, 512], f32, name="ind")
                nc.vector.tensor_tensor(out=ind[:], in0=d2["e"][:, js],
                                        in1=d2["a"][:, js], op=ALU.is_le)
                sel = tail.tile([2, 512], f32, name="sel")
                nc.vector.tensor_tensor(out=sel[:], in0=pair["mu1"][:],
                                        in1=pair["mu2"][:], op=ALU.subtract)
                nc.vector.tensor_tensor(out=sel[:], in0=sel[:], in1=ind[:], op=ALU.mult)
                nc.vector.tensor_tensor(out=sel[:], in0=sel[:], in1=pair["mu2"][:],
                                        op=ALU.add)
                mu12 = tail.tile([2, 512], f32, name="mu12")
                nc.vector.tensor_tensor(out=mu12[:], in0=pair["mu1"][:],
                                        in1=pair["mu2"][:], op=ALU.max)
                nc.vector.tensor_tensor(out=mu12[:], in0=mu12[:], in1=pair["cc"][:],
                                        op=ALU.add)
                delta = tail.tile([2, 512], f32, name="delta")
                nc.vector.tensor_tensor(out=delta[:], in0=PT["a"][:, js],
                                        in1=PT["e"][:, js], op=ALU.subtract)
                res = tail.tile([2, 512], f32, name="res")
                nc.vector.tensor_tensor(out=res[:], in0=delta[:], in1=mu12[:],
                                        op=ALU.mult)
                r1 = tail.tile([2, 512], f32, name="r1")
                nc.vector.tensor_tensor(out=r1[:], in0=PT["e"][:, js], in1=sel[:],
                                        op=ALU.mult)
                nc.vector.tensor_tensor(out=res[:], in0=res[:], in1=r1[:], op=ALU.add)

                # out0 = sigmoid(res0 - res1), out1 = sigmoid(res1 - res0)
                dps = psB.tile([128, 512], f32, tag="ps2", name=f"dps{j}")
                nc.tensor.matmul(dps[0:1, :], P("pm1"), res[:], start=True, stop=True)
                o0 = tail.tile([1, 512], f32, name="o0")
                o1 = tail.tile([1, 512], f32, name="o1")
                nc.scalar.activation(out=o0[:], in_=dps[0:1, :], func=AF.Sigmoid,
                                     bias=0.0, scale=1.0)
                nc.scalar.activation(out=o1[:], in_=dps[0:1, :], func=AF.Sigmoid,
                                     bias=0.0, scale=-1.0)
                nc.sync.dma_start(out[0:1, js], o0[:])
                nc.sync.dma_start(out[1:2, js], o1[:])

    nc.compile()
    return nc


def _np(a):
    return np.asarray(a, dtype=np.float32)


def _host_pack(params):
    """Build spack [128, S] + W1 packs from the params pytree."""
    cols, S = _layout()
    spack = np.zeros((128, S), dtype=np.float32)

    def put(name, arr):
        r, o, n = cols[name]
        arr = np.asarray(arr, dtype=np.float32)
        assert arr.shape == (r, n), (name, arr.shape, (r, n))
        spack[0:r, o:o + n] = arr

    pe, pa = params["eeg_ext"], params["aud_ext"]
    ble, bla = params["eeg_bl"], params["aud_bl"]
    sng, cmp_ = params["single"], params["comp"]

    put("W2e", _np(pe["w2b2"][0])); put("W2a", _np(pa["w2b2"][0]))
    put("b2e", _np(pe["w2b2"][1])[:, None]); put("b2a", _np(pa["w2b2"][1])[:, None])
    put("b1e", _np(pe["w1b1"][1])[:, None]); put("b1a", _np(pa["w1b1"][1])[:, None])

    for m, bl in (("e", ble), ("a", bla)):
        put("BW1" + m, _np(bl["w1b1"][0])); put("Bb1" + m, _np(bl["w1b1"][1])[:, None])
        put("BW2" + m, _np(bl["w2b2"][0])); put("Bb2" + m, _np(bl["w2b2"][1])[:, None])
        w3, b3 = _np(bl["w3b3"][0]), _np(bl["w3b3"][1])
        w3d = w3[:, 0] - w3[:, 1]
        b3d = float(b3[0] - b3[1])
        put("W3d2" + m, np.stack([w3d, -w3d], axis=1))
        put("b3d2" + m, np.array([[b3d], [-b3d]], dtype=np.float32))

    # fuzzy L1: fold Pe1 = 1 - Pe0 / Pa1 = 1 - Pa0 into weights+bias.
    W1s, b1s = _np(sng["w1b1"][0]), _np(sng["w1b1"][1])   # [6, 64], [64]
    W1c, b1c = _np(cmp_["w1b1"][0]), _np(cmp_["w1b1"][1])
    eff_s = np.stack([W1s[0], W1s[1] - W1s[2], W1s[3], W1s[4] - W1s[5]])  # [4, 64]
    eff_c = np.stack([W1c[0], W1c[1] - W1c[2], W1c[3], W1c[4] - W1c[5]])
    b1s_eff = b1s + W1s[2] + W1s[5]
    b1c_eff = b1c + W1c[2] + W1c[5]
    for r in range(4):
        put(f"U1r{r}", np.concatenate([eff_s[r], eff_c[r]])[None, :])   # [1, 128]
    put("b1cat", np.concatenate([b1s_eff, b1c_eff])[:, None])

    W2s, b2s = _np(sng["w2b2"][0]), _np(sng["w2b2"][1])   # [64, 4], [4]
    W2c, b2c = _np(cmp_["w2b2"][0]), _np(cmp_["w2b2"][1])  # [64, 2], [2]
    z = np.zeros((64, 2), dtype=np.float32)
    put("W2s1", np.concatenate([W2s[:, 0:2], z], axis=0))
    put("W2s2", np.concatenate([W2s[:, 2:4], z], axis=0))
    put("W2cp", np.concatenate([z, W2c], axis=0))
    put("b2s1", b2s[0:2, None]); put("b2s2", b2s[2:4, None]); put("b2cp", b2c[:, None])
    put("pm1", np.array([[1.0], [-1.0]], dtype=np.float32))

    def w1pack(w):  # [4096, 128] -> [128, 32*128] with [p, k*128+f] = W[k*128+p, f]
        w = np.asarray(w, dtype=np.float32)
        return np.ascontiguousarray(
            w.reshape(_KC, 128, 128).transpose(1, 0, 2).reshape(128, _KD))

    return spack, w1pack(pe["w1b1"][0]), w1pack(pa["w1b1"][0])


def kernel(eeg_input, audio_input, labels, params):
    from concourse import bass_utils

    import os
    mode = os.environ.get("MF2_MM_DTYPE", "float32")
    if _CACHE.get("mode") != mode:
        _CACHE["nc"] = _build_module(mode)
        _CACHE["mode"] = mode
    nc = _CACHE["nc"]

    spack, w1e_p, w1a_p = _host_pack(params)
    eeg = np.asarray(eeg_input, dtype=np.float32)
    aud = np.asarray(audio_input, dtype=np.float32)
    lab_f = np.asarray(labels).astype(np.float32)
    lab12 = np.stack([lab_f, 1.0 - 2.0 * lab_f], axis=0).astype(np.float32)  # [2, B]

    in_maps = []
    for c in range(_NCORES):
        sl = slice(c * _BL, (c + 1) * _BL)
        in_maps.append({
            "xe": np.ascontiguousarray(eeg[sl].T),
            "xa": np.ascontiguousarray(aud[sl].T),
            "w1e": w1e_p,
            "w1a": w1a_p,
            "lab12": np.ascontiguousarray(lab12[:, sl]).reshape(1, 2 * _BL),
            "spk": spack,
        })

    r = bass_utils.run_bass_kernel_spmd(nc, in_maps, core_ids=list(range(_NCORES)))
    out = np.concatenate([r.results[c]["out"].T for c in range(_NCORES)], axis=0)
    return out.astype(np.float32)
